# revision 30
# baseline (speedup 1.0000x reference)
"""Bass/Trainium2 kernel for nn_Attend (masked+biased multi-head attention).

Problem (hardcoded): b=2, n=2048, d_model=512, h=8 heads, d=64.
  out[b,h,i,:] = softmax_j(q_h[b,i]·k_h[b,j]*scale masked + bias[h,i,j]) @ v_h[b]

Sharding: head-parallel across the 8 NeuronCores (core c <-> head c), both
batches on every core.  No cross-core communication.

Per-core device algorithm (everything stored "transposed", j on partitions):
  S_T[j,i]   = sum_d kT[d,j] * qT_scaled[d,i]          (PE, K=64)
  S_T       += bias_T[j,i]                             (PE identity-matmul accumulate)
  S_T       += -1e9 * mask_T[j,i]                      (DVE scalar_tensor_tensor)
  E_T        = exp(S_T)                                (ACT, PSUM->SBUF)
  outT[d,i], Z[i] = sum_j v_aug[j,d-or-ones] * E_T[j,i] (PE, accumulate over j)
  out[i,d]   = transpose(outT)[i,d] / Z[i]             (PE transpose + DVE)

The j-loop streams bias_T/mask_T tiles (the dominant HBM traffic) through
SBUF once; this problem is HBM-bandwidth bound on the bias matrix.
"""

import os
from contextlib import ExitStack

import numpy as np

B = 2
N = 2048
DM = 512
H = 8
D = 64  # head dim

JB = 128          # j rows per block (partition dim)
NJ = N // JB      # 16 j blocks
IC = 512          # i columns per matmul (one PSUM bank of fp32)
IH = 1024         # i columns per exp op (2 banks)
NEG = -1.0e9
BIG = 4096.0

# --- tunables ---------------------------------------------------------------
CFG = {
    "e_dtype": os.environ.get("ATT_E_DTYPE", "bf16"),      # f32 | bf16
    "v_dtype": os.environ.get("ATT_V_DTYPE", "bf16"),      # f32 | bf16
    "mask_mode": os.environ.get("ATT_MASK_MODE", "stt"),  # stt | postmul
    "mm_dtype": os.environ.get("ATT_MM_DTYPE", "bf16"),   # f32 | f32r | bf16
    "bias_on_pe": os.environ.get("ATT_BIAS_ON_PE", "1") == "1",
    "s_bufs": int(os.environ.get("ATT_S_BUFS", "4")),
    "gps_frac8": int(os.environ.get("ATT_GPS_FRAC8", "3")),
    "in_bufs": int(os.environ.get("ATT_IN_BUFS", "6")),
}


def _dt(mybir, name):
    return {"f32": mybir.dt.float32, "bf16": mybir.dt.bfloat16}[name]


def build_program(scale: float, cfg=None):
    """Build the single-core SPMD Bass program (same NEFF on all 8 cores)."""
    import concourse.bass as bass
    import concourse.tile as tile
    from concourse import bacc, mybir

    cfg = dict(CFG, **(cfg or {}))
    e_dt = _dt(mybir, cfg["e_dtype"])
    v_dt = _dt(mybir, cfg["v_dtype"])
    f32 = mybir.dt.float32
    u8 = mybir.dt.uint8
    Exp = mybir.ActivationFunctionType.Exp
    Copy = mybir.ActivationFunctionType.Copy

    nc = bacc.Bacc()
    mdt = {"f32r": mybir.dt.float32r, "bf16": mybir.dt.bfloat16,
           "f32": f32}[cfg["mm_dtype"]]

    qT = nc.declare_dram_parameter("qT", [B, 128, N], mdt, isOutput=False)
    kT = nc.declare_dram_parameter("kT", [B, 128, N], mdt, isOutput=False)
    vh = nc.declare_dram_parameter("v", [B, N, D], v_dt, isOutput=False)
    biasT = nc.declare_dram_parameter("biasT", [N, N], mdt, isOutput=False)
    maskT = nc.declare_dram_parameter("maskT", [B, N, N], e_dt, isOutput=False)
    ident = nc.declare_dram_parameter("ident", [128, 128], f32, isOutput=False)
    # identity scaled by 1/scale: the PE bias-add injects bias/scale, and the
    # exp applies the scale to the whole pre-activation sum
    idents = nc.declare_dram_parameter("idents", [128, 128], mdt, isOutput=False)
    out = nc.declare_dram_parameter("out", [B, N, D], f32, isOutput=True)

    with ExitStack() as ctx:
        tc = ctx.enter_context(tile.TileContext(nc))
        singles = ctx.enter_context(tc.tile_pool(name="singles", bufs=1))
        ins = ctx.enter_context(tc.tile_pool(name="ins", bufs=cfg["in_bufs"]))
        biasp = ctx.enter_context(tc.tile_pool(name="biasp", bufs=NJ))
        invs = ctx.enter_context(tc.tile_pool(name="invs", bufs=cfg["in_bufs"]))
        es = ctx.enter_context(tc.tile_pool(name="es", bufs=4))
        drains = ctx.enter_context(tc.tile_pool(name="drains", bufs=2))
        smalls = ctx.enter_context(tc.tile_pool(name="smalls", bufs=8))
        spool = ctx.enter_context(tc.tile_pool(name="spool", bufs=cfg["s_bufs"], space="PSUM"))
        opool = ctx.enter_context(tc.tile_pool(name="opool", bufs=1, space="PSUM"))

        # ---- one-time loads -------------------------------------------------
        ident_sb = singles.tile([128, 128], f32, tag="ident")
        nc.sync.dma_start(out=ident_sb, in_=ident[:, :])
        idents_sb = singles.tile([128, 128], mdt, tag="idents")
        nc.sync.dma_start(out=idents_sb, in_=idents[:, :])

        # q/k arrive host-padded to 128 contraction rows (zeros below row 64):
        # full-K matmuls keep the PE activity monitor warm at no stream cost
        qT_sb, kT_sb = [], []
        for b in range(B):
            qb = singles.tile([128, N], mdt, name=f"qTs{b}", tag=f"qT{b}")
            nc.sync.dma_start(out=qb, in_=qT[b])
            qT_sb.append(qb)
            kb = singles.tile([128, N], mdt, name=f"kTs{b}", tag=f"kT{b}")
            nc.sync.dma_start(out=kb, in_=kT[b])
            kT_sb.append(kb)

        # persistent v slots: the ones-column is written once per slot
        NVS = 6
        v_slots = []
        for s in range(NVS):
            vt = singles.tile([JB, D + 1], v_dt, name=f"vslot{s}", tag=f"vslot{s}")
            nc.vector.memset(vt[:, D:D + 1], 1.0)
            v_slots.append(vt)

        state = {}
        bias_tiles = {}

        def emit_iter(b, j):
            st = state[b]
            # bias has no batch dim: load once (b=0), stay resident for b=1
            if j in bias_tiles:
                bias_sb = bias_tiles[j]
            else:
                bias_sb = biasp.tile([JB, N], mdt, name="bias_sb", tag="bias")
                nc.sync.dma_start(out=bias_sb, in_=biasT[j * JB:(j + 1) * JB, :])
                bias_tiles[j] = bias_sb
            mask_sb = ins.tile([JB, N], e_dt, name="mask_sb", tag="mask")
            nc.sync.dma_start(out=mask_sb, in_=maskT[b, j * JB:(j + 1) * JB, :])
            invm = mask_sb

            v_aug = v_slots[(b * NJ + j) % NVS]
            nc.sync.dma_start(out=v_aug[:, 0:D], in_=vh[b, j * JB:(j + 1) * JB, :])

            e_sb = es.tile([JB, N], e_dt, name="e_sb", tag="e")
            for c in range(N // IC):
                csl = bass.ts(c, IC)
                s_ps = spool.tile([JB, IC], f32, name="s_ps", tag="s")
                nc.tensor.matmul(
                    s_ps,
                    lhsT=kT_sb[b][:, j * JB:(j + 1) * JB],
                    rhs=qT_sb[b][:, csl],
                    start=True, stop=False,
                )
                nc.tensor.matmul(
                    s_ps,
                    lhsT=idents_sb,
                    rhs=bias_sb[:, csl],
                    start=False, stop=True,
                )
                nc.scalar.activation(out=e_sb[:, csl], in_=s_ps, func=Exp,
                                     scale=float(scale))
                nc.vector.tensor_tensor(
                    out=e_sb[:, csl], in0=e_sb[:, csl],
                    in1=invm[:, csl], op=mybir.AluOpType.mult,
                )
                if st["prev"] is not None:
                    nc.tensor.matmul(
                        st["pv"][c],
                        lhsT=st["prev"][0],
                        rhs=st["prev"][1][:, csl],
                        start=(j == 1), stop=False,
                    )
            st["prev"] = (v_aug, e_sb)

        def emit_final_pv(b):
            st = state[b]
            for c in range(N // IC):
                nc.tensor.matmul(
                    st["pv"][c],
                    lhsT=st["prev"][0],
                    rhs=st["prev"][1][:, bass.ts(c, IC)],
                    start=False, stop=True,
                )

        def emit_drain_copies(b):
            st = state[b]
            ot_sb = drains.tile([D + 1, N], f32, name="ot_sb", tag="ot")
            st["ot"] = ot_sb
            for c in range(N // IC):
                nc.scalar.activation(out=ot_sb[:, bass.ts(c, IC)], in_=st["pv"][c], func=Copy)

        def emit_drain(b):
            st = state[b]
            ot_sb = st["ot"]
            ostage = drains.tile([128, N // 128 * D], f32, name="ostage", tag="ostage")
            for t in range(N // 128):
                t_ps = spool.tile([128, D + 1], f32, name="t_ps", tag="s")
                nc.tensor.transpose(
                    t_ps, ot_sb[:, t * 128:(t + 1) * 128], ident_sb[0:D + 1, 0:D + 1],
                )
                rz = smalls.tile([128, 1], f32, name="rz", tag="rz")
                nc.vector.reciprocal(rz, t_ps[:, D:D + 1])
                nc.vector.tensor_scalar_mul(ostage[:, bass.ts(t, D)], t_ps[:, 0:D], rz)

            nc.sync.dma_start(
                out=out[b].rearrange("(t p) d -> p t d", p=128),
                in_=ostage.rearrange("p (t d) -> p t d", d=D),
            )

        def start_batch(b):
            state[b] = {
                "pv": [opool.tile([D + 1, IC], f32, name=f"pv{b}_{ic}", tag=f"pv{ic}")
                       for ic in range(N // IC)],
                "prev": None,
            }

        # batch 0, then overlap batch-0's drain with batch-1's first
        # iteration so the PE never idles long enough to re-throttle
        start_batch(0)
        for j in range(NJ):
            emit_iter(0, j)
        emit_final_pv(0)
        emit_drain_copies(0)
        start_batch(1)
        emit_iter(1, 0)
        emit_drain(0)
        for j in range(1, NJ):
            emit_iter(1, j)
        emit_final_pv(1)
        emit_drain_copies(1)
        emit_drain(1)

    nc.compile()
    return nc


_PROG_CACHE = {}


def _get_program(scale: float):
    key = (round(float(scale), 9), tuple(sorted(CFG.items())))
    if key not in _PROG_CACHE:
        _PROG_CACHE[key] = build_program(float(scale))
    return _PROG_CACHE[key]


_SCALE_HOLDER = [0.125]


def _kpad(t, np_dt):
    import numpy as _np
    p = _np.zeros((t.shape[0], 128, t.shape[2]), dtype=np_dt)
    p[:, 0:t.shape[1], :] = t.astype(np_dt)
    return p


def make_in_maps(q, k, v, mask, bias):
    import ml_dtypes
    mm_np = {"f32": np.float32, "f32r": np.float32,
             "bf16": ml_dtypes.bfloat16}[CFG["mm_dtype"]]
    v_np = {"f32": np.float32, "bf16": ml_dtypes.bfloat16}[CFG["v_dtype"]]
    e_np = {"f32": np.float32, "bf16": ml_dtypes.bfloat16}[CFG["e_dtype"]]
    q = np.asarray(q, dtype=np.float32)
    k = np.asarray(k, dtype=np.float32)
    v = np.asarray(v, dtype=np.float32)
    mask_u8 = np.asarray(mask).astype(np.uint8)  # (B,1,N,N), True==masked
    bias = np.asarray(bias, dtype=np.float32)    # (1,H,N,N)
    eye = np.eye(128, dtype=np.float32)
    scale_f = float(np.asarray(_SCALE_HOLDER[0]))
    eyes = (eye / scale_f).astype(mm_np)
    eye_mm = eye.astype(mm_np)

    in_maps = []
    for h in range(H):
        sl = slice(h * D, (h + 1) * D)
        in_maps.append({
            "qT": _kpad(q[:, :, sl].transpose(0, 2, 1), mm_np),
            "kT": _kpad(k[:, :, sl].transpose(0, 2, 1), mm_np),
            "v": np.ascontiguousarray(v[:, :, sl]).astype(v_np),
            "biasT": np.ascontiguousarray(bias[0, h].T).astype(mm_np),
            "maskT": np.ascontiguousarray((1 - mask_u8)[:, 0].transpose(0, 2, 1)).astype(e_np),
            "ident": eye,
            "idents": eyes,
        })
    return in_maps


def run(q, k, v, scale, mask, bias, trace=False, trace_kwargs=None):
    from concourse.bass_utils import run_bass_kernel_spmd

    _SCALE_HOLDER[0] = float(np.asarray(scale))
    nc = _get_program(float(np.asarray(scale)))
    in_maps = make_in_maps(q, k, v, mask, bias)
    res = run_bass_kernel_spmd(
        nc, in_maps, core_ids=list(range(H)),
        trace=trace, **(trace_kwargs or {}),
    )
    outs = [np.asarray(res.results[h]["out"]) for h in range(H)]
    full = np.stack(outs, axis=1).astype(np.float32)  # (B, H, N, D)
    return full, res


def kernel(q, k, v, scale, mask, bias):
    full, _ = run(q, k, v, scale, mask, bias, trace=False)
    return full


# revision 31
# speedup vs baseline: 1.0286x; 1.0286x over previous
"""Bass/Trainium2 kernel for nn_Attend (masked+biased multi-head attention).

Problem (hardcoded): b=2, n=2048, d_model=512, h=8 heads, d=64.
  out[b,h,i,:] = softmax_j(q_h[b,i]·k_h[b,j]*scale masked + bias[h,i,j]) @ v_h[b]

Sharding: head-parallel across the 8 NeuronCores (core c <-> head c), both
batches on every core.  No cross-core communication.

Per-core device algorithm (everything stored "transposed", j on partitions):
  S_T[j,i]   = sum_d kT[d,j] * qT_scaled[d,i]          (PE, K=64)
  S_T       += bias_T[j,i]                             (PE identity-matmul accumulate)
  S_T       += -1e9 * mask_T[j,i]                      (DVE scalar_tensor_tensor)
  E_T        = exp(S_T)                                (ACT, PSUM->SBUF)
  outT[d,i], Z[i] = sum_j v_aug[j,d-or-ones] * E_T[j,i] (PE, accumulate over j)
  out[i,d]   = transpose(outT)[i,d] / Z[i]             (PE transpose + DVE)

The j-loop streams bias_T/mask_T tiles (the dominant HBM traffic) through
SBUF once; this problem is HBM-bandwidth bound on the bias matrix.
"""

import os
from contextlib import ExitStack

import numpy as np

B = 2
N = 2048
DM = 512
H = 8
D = 64  # head dim

JB = 128          # j rows per block (partition dim)
NJ = N // JB      # 16 j blocks
IC = 512          # i columns per matmul (one PSUM bank of fp32)
IH = 1024         # i columns per exp op (2 banks)
NEG = -1.0e9
BIG = 4096.0

# --- tunables ---------------------------------------------------------------
CFG = {
    "e_dtype": os.environ.get("ATT_E_DTYPE", "bf16"),      # f32 | bf16
    "v_dtype": os.environ.get("ATT_V_DTYPE", "bf16"),      # f32 | bf16
    "mask_mode": os.environ.get("ATT_MASK_MODE", "stt"),  # stt | postmul
    "mm_dtype": os.environ.get("ATT_MM_DTYPE", "bf16"),   # f32 | f32r | bf16
    "bias_on_pe": os.environ.get("ATT_BIAS_ON_PE", "1") == "1",
    "s_bufs": int(os.environ.get("ATT_S_BUFS", "4")),
    "gps_frac8": int(os.environ.get("ATT_GPS_FRAC8", "3")),
    "in_bufs": int(os.environ.get("ATT_IN_BUFS", "6")),
}


def _dt(mybir, name):
    return {"f32": mybir.dt.float32, "bf16": mybir.dt.bfloat16}[name]


def build_program(scale: float, cfg=None):
    """Build the single-core SPMD Bass program (same NEFF on all 8 cores)."""
    import concourse.bass as bass
    import concourse.tile as tile
    from concourse import bacc, mybir

    cfg = dict(CFG, **(cfg or {}))
    e_dt = _dt(mybir, cfg["e_dtype"])
    v_dt = _dt(mybir, cfg["v_dtype"])
    f32 = mybir.dt.float32
    u8 = mybir.dt.uint8
    Exp = mybir.ActivationFunctionType.Exp
    Copy = mybir.ActivationFunctionType.Copy

    nc = bacc.Bacc()
    mdt = {"f32r": mybir.dt.float32r, "bf16": mybir.dt.bfloat16,
           "f32": f32}[cfg["mm_dtype"]]

    qT = nc.declare_dram_parameter("qT", [B, 128, N], mdt, isOutput=False)
    kT = nc.declare_dram_parameter("kT", [B, 128, N], mdt, isOutput=False)
    vh = nc.declare_dram_parameter("v", [B, N, D], v_dt, isOutput=False)
    biasT = nc.declare_dram_parameter("biasT", [N, N], mdt, isOutput=False)
    maskT = nc.declare_dram_parameter("maskT", [B, N, N], e_dt, isOutput=False)
    ident = nc.declare_dram_parameter("ident", [128, 128], f32, isOutput=False)
    # identity scaled by 1/scale: the PE bias-add injects bias/scale, and the
    # exp applies the scale to the whole pre-activation sum
    idents = nc.declare_dram_parameter("idents", [128, 128], mdt, isOutput=False)
    out = nc.declare_dram_parameter("out", [B, N, D], f32, isOutput=True)

    with ExitStack() as ctx:
        tc = ctx.enter_context(tile.TileContext(nc))
        singles = ctx.enter_context(tc.tile_pool(name="singles", bufs=1))
        ins = ctx.enter_context(tc.tile_pool(name="ins", bufs=cfg["in_bufs"]))
        biasp = ctx.enter_context(tc.tile_pool(name="biasp", bufs=NJ))
        invs = ctx.enter_context(tc.tile_pool(name="invs", bufs=cfg["in_bufs"]))
        es = ctx.enter_context(tc.tile_pool(name="es", bufs=6))
        drains = ctx.enter_context(tc.tile_pool(name="drains", bufs=2))
        smalls = ctx.enter_context(tc.tile_pool(name="smalls", bufs=8))
        spool = ctx.enter_context(tc.tile_pool(name="spool", bufs=cfg["s_bufs"], space="PSUM"))
        opool = ctx.enter_context(tc.tile_pool(name="opool", bufs=1, space="PSUM"))

        # ---- one-time loads -------------------------------------------------
        ident_sb = singles.tile([128, 128], f32, tag="ident")
        nc.sync.dma_start(out=ident_sb, in_=ident[:, :])
        idents_sb = singles.tile([128, 128], mdt, tag="idents")
        nc.sync.dma_start(out=idents_sb, in_=idents[:, :])

        # q/k arrive host-padded to 128 contraction rows (zeros below row 64):
        # full-K matmuls keep the PE activity monitor warm at no stream cost
        qT_sb, kT_sb = [], []
        for b in range(B):
            qb = singles.tile([128, N], mdt, name=f"qTs{b}", tag=f"qT{b}")
            nc.sync.dma_start(out=qb, in_=qT[b])
            qT_sb.append(qb)
            kb = singles.tile([128, N], mdt, name=f"kTs{b}", tag=f"kT{b}")
            nc.sync.dma_start(out=kb, in_=kT[b])
            kT_sb.append(kb)

        # persistent v slots: the ones-column is written once per slot
        NVS = 6
        v_slots = []
        for s in range(NVS):
            vt = singles.tile([JB, D + 1], v_dt, name=f"vslot{s}", tag=f"vslot{s}")
            nc.vector.memset(vt[:, D:D + 1], 1.0)
            v_slots.append(vt)

        state = {}
        bias_tiles = {}

        def emit_iter(b, j):
            st = state[b]
            # bias has no batch dim: load once (b=0), stay resident for b=1
            if j in bias_tiles:
                bias_sb = bias_tiles[j]
            else:
                bias_sb = biasp.tile([JB, N], mdt, name="bias_sb", tag="bias")
                nc.sync.dma_start(out=bias_sb, in_=biasT[j * JB:(j + 1) * JB, :])
                bias_tiles[j] = bias_sb
            mask_sb = ins.tile([JB, N], e_dt, name="mask_sb", tag="mask")
            nc.sync.dma_start(out=mask_sb, in_=maskT[b, j * JB:(j + 1) * JB, :])
            invm = mask_sb

            v_aug = v_slots[(b * NJ + j) % NVS]
            nc.sync.dma_start(out=v_aug[:, 0:D], in_=vh[b, j * JB:(j + 1) * JB, :])

            e_sb = es.tile([JB, N], e_dt, name="e_sb", tag="e")
            for c in range(N // IC):
                csl = bass.ts(c, IC)
                s_ps = spool.tile([JB, IC], f32, name="s_ps", tag="s")
                nc.tensor.matmul(
                    s_ps,
                    lhsT=kT_sb[b][:, j * JB:(j + 1) * JB],
                    rhs=qT_sb[b][:, csl],
                    start=True, stop=False,
                )
                nc.tensor.matmul(
                    s_ps,
                    lhsT=idents_sb,
                    rhs=bias_sb[:, csl],
                    start=False, stop=True,
                )
                nc.scalar.activation(out=e_sb[:, csl], in_=s_ps, func=Exp,
                                     scale=float(scale))
                nc.vector.tensor_tensor(
                    out=e_sb[:, csl], in0=e_sb[:, csl],
                    in1=invm[:, csl], op=mybir.AluOpType.mult,
                )
                if st["prev"] is not None:
                    nc.tensor.matmul(
                        st["pv"][c],
                        lhsT=st["prev"][0],
                        rhs=st["prev"][1][:, csl],
                        start=(j == 1), stop=False,
                    )
            st["prev"] = (v_aug, e_sb)

        def emit_final_pv(b):
            st = state[b]
            for c in range(N // IC):
                nc.tensor.matmul(
                    st["pv"][c],
                    lhsT=st["prev"][0],
                    rhs=st["prev"][1][:, bass.ts(c, IC)],
                    start=False, stop=True,
                )

        def emit_drain_copies(b):
            st = state[b]
            ot_sb = drains.tile([D + 1, N], f32, name="ot_sb", tag="ot")
            st["ot"] = ot_sb
            for c in range(N // IC):
                nc.scalar.activation(out=ot_sb[:, bass.ts(c, IC)], in_=st["pv"][c], func=Copy)

        def emit_drain(b):
            st = state[b]
            ot_sb = st["ot"]
            ostage = drains.tile([128, N // 128 * D], f32, name="ostage", tag="ostage")
            for t in range(N // 128):
                t_ps = spool.tile([128, D + 1], f32, name="t_ps", tag="s")
                nc.tensor.transpose(
                    t_ps, ot_sb[:, t * 128:(t + 1) * 128], ident_sb[0:D + 1, 0:D + 1],
                )
                rz = smalls.tile([128, 1], f32, name="rz", tag="rz")
                nc.vector.reciprocal(rz, t_ps[:, D:D + 1])
                nc.vector.tensor_scalar_mul(ostage[:, bass.ts(t, D)], t_ps[:, 0:D], rz)

            nc.sync.dma_start(
                out=out[b].rearrange("(t p) d -> p t d", p=128),
                in_=ostage.rearrange("p (t d) -> p t d", d=D),
            )

        def start_batch(b):
            state[b] = {
                "pv": [opool.tile([D + 1, IC], f32, name=f"pv{b}_{ic}", tag=f"pv{ic}")
                       for ic in range(N // IC)],
                "prev": None,
            }

        # batch 0, then overlap batch-0's drain with batch-1's first
        # iteration so the PE never idles long enough to re-throttle
        start_batch(0)
        for j in range(NJ):
            emit_iter(0, j)
        emit_final_pv(0)
        emit_drain_copies(0)
        start_batch(1)
        emit_iter(1, 0)
        emit_drain(0)
        for j in range(1, NJ):
            emit_iter(1, j)
        emit_final_pv(1)
        emit_drain_copies(1)
        emit_drain(1)

    nc.compile()
    return nc


_PROG_CACHE = {}


def _get_program(scale: float):
    key = (round(float(scale), 9), tuple(sorted(CFG.items())))
    if key not in _PROG_CACHE:
        _PROG_CACHE[key] = build_program(float(scale))
    return _PROG_CACHE[key]


_SCALE_HOLDER = [0.125]


def _kpad(t, np_dt):
    import numpy as _np
    p = _np.zeros((t.shape[0], 128, t.shape[2]), dtype=np_dt)
    p[:, 0:t.shape[1], :] = t.astype(np_dt)
    return p


def make_in_maps(q, k, v, mask, bias):
    import ml_dtypes
    mm_np = {"f32": np.float32, "f32r": np.float32,
             "bf16": ml_dtypes.bfloat16}[CFG["mm_dtype"]]
    v_np = {"f32": np.float32, "bf16": ml_dtypes.bfloat16}[CFG["v_dtype"]]
    e_np = {"f32": np.float32, "bf16": ml_dtypes.bfloat16}[CFG["e_dtype"]]
    q = np.asarray(q, dtype=np.float32)
    k = np.asarray(k, dtype=np.float32)
    v = np.asarray(v, dtype=np.float32)
    mask_u8 = np.asarray(mask).astype(np.uint8)  # (B,1,N,N), True==masked
    bias = np.asarray(bias, dtype=np.float32)    # (1,H,N,N)
    eye = np.eye(128, dtype=np.float32)
    scale_f = float(np.asarray(_SCALE_HOLDER[0]))
    eyes = (eye / scale_f).astype(mm_np)
    eye_mm = eye.astype(mm_np)

    in_maps = []
    for h in range(H):
        sl = slice(h * D, (h + 1) * D)
        in_maps.append({
            "qT": _kpad(q[:, :, sl].transpose(0, 2, 1), mm_np),
            "kT": _kpad(k[:, :, sl].transpose(0, 2, 1), mm_np),
            "v": np.ascontiguousarray(v[:, :, sl]).astype(v_np),
            "biasT": np.ascontiguousarray(bias[0, h].T).astype(mm_np),
            "maskT": np.ascontiguousarray((1 - mask_u8)[:, 0].transpose(0, 2, 1)).astype(e_np),
            "ident": eye,
            "idents": eyes,
        })
    return in_maps


def run(q, k, v, scale, mask, bias, trace=False, trace_kwargs=None):
    from concourse.bass_utils import run_bass_kernel_spmd

    _SCALE_HOLDER[0] = float(np.asarray(scale))
    nc = _get_program(float(np.asarray(scale)))
    in_maps = make_in_maps(q, k, v, mask, bias)
    res = run_bass_kernel_spmd(
        nc, in_maps, core_ids=list(range(H)),
        trace=trace, **(trace_kwargs or {}),
    )
    outs = [np.asarray(res.results[h]["out"]) for h in range(H)]
    full = np.stack(outs, axis=1).astype(np.float32)  # (B, H, N, D)
    return full, res


def kernel(q, k, v, scale, mask, bias):
    full, _ = run(q, k, v, scale, mask, bias, trace=False)
    return full
